# revision 26
# baseline (speedup 1.0000x reference)
"""Trainium2 Bass kernel for nn_BilinearModule (16,256,64,64 bilinear pooling).

Math (per image):
  y   = relu(bn1(w1 @ x + b1))                       # (32, 4096)
  packed[t] = y[r_t] * y[c_t]  for 528 lower-tri pairs
  out = relu(bn2(w2 @ packed + b2))                  # (256, 4096)

Strategy (pure data parallel over batch, 2 images per core, 8 cores):
  - all matmul operands bf16 (x cast host-side); fp32 PSUM accumulation and
    fp32 BN math; output stored bf16 and widened to fp32 on the host
    (rel err ~4.9e-3, same as fp32 store).
  - mm1 with M-replicated weights -> psum; fused BN1+ReLU on ACT -> yrep bf16
    (4 identical copies of the 32 channels across 128 partitions).
  - The 528 pair-products = rotations r=0..16 of the 32 channels. r=0..15
    (512 products) come from 4 channel-rotation matmuls (K=32, 4 rotations
    per 128-row tile, one per PE row-strip so all 4 overlap in the array).
    The 16 r=16 products need no PE pass at all: p16 = yrep[0:16]*yrep[16:32]
    on DVE (partition-shifted slices of the replicated tile).
  - Products: DVE tensor_mul reads rotated tiles straight from PSUM for
    j=1..3; tile 0 goes ACT copy -> GpSimd so DVE stays under the PE period.
  - mm2 = per output chunk: K=16 (p16) + 4x K=128 bf16 passes with host-side
    permuted w2; fused BN2+ReLU on ACT writes a whole-image SBUF buffer.
  - DMA plan: ~17 coarse transfers instead of ~60 (SP DGE setup is ~0.6us
    per dma_start). First x quarter rides the ACT hardware DGE as ACT's
    first instructions; consts are packed into 3 dram params; outputs
    buffered in SBUF and flushed in halves/quarters per (img, chunk).
  - 4 DMA-free warmup matmuls (iota-generated toggling data) bridge the
    preamble so the HAM clock-gate opens before the first real window.
  - 2-deep software pipeline: PE stream per window is
    mm1(w) | mm2_m0(w-2) | sel(w) | mm2_m1(w-2), so BN/products hide
    behind matmuls with a full window of slack.
"""

import numpy as np

import concourse.bass as bass
import concourse.mybir as mybir
from concourse import tile
from concourse.bass_utils import run_bass_kernel_spmd

F32 = mybir.dt.float32
BF16 = mybir.dt.bfloat16
AF = mybir.ActivationFunctionType

N_CORES = 8
B, CIN, H, W = 16, 256, 64, 64
NPIX = H * W                     # 4096
IMG_PER_CORE = B // N_CORES      # 2
CMID = 32
COUT = 256
FB = 512                         # pixel window (psum-bank sized)
NWIN = NPIX // FB                # 8 windows per image
EPS = 1e-5

# cb1 columns: [w1a | w1b | perm(4x128)]
CB1_W1A, CB1_W1B, CB1_PERM = 0, 128, 256
CB1_COLS = 256 + 4 * 128
# cb2 columns: [w2p tiles j=0..3 (256 each) | w2p16 (256)]
CB2_P16 = 4 * COUT
CB2_COLS = 5 * COUT

# output flush points: (img, win, m) -> pixel range [a, b)
_FLUSH = {}
for _m in range(2):
    _FLUSH[(0, 3, _m)] = (0, 2048)
    _FLUSH[(0, 7, _m)] = (2048, 4096)
    _FLUSH[(1, 3, _m)] = (0, 2048)
    _FLUSH[(1, 5, _m)] = (2048, 3072)
    _FLUSH[(1, 6, _m)] = (3072, 3584)
    _FLUSH[(1, 7, _m)] = (3584, 4096)

_ctr = [0]


def _split_multi_waits(nc):
    """This container's walrus supports one sync-wait per instruction; split
    extras onto NOP carriers on the same engine."""
    for f in nc.m.functions:
        for blk in f.blocks:
            insts = blk.instructions
            if not any(
                i.sync_info is not None and len(i.sync_info.on_wait) > 1
                for i in insts
            ):
                continue
            new = []
            for inst in insts:
                si = inst.sync_info
                if si is not None and len(si.on_wait) > 1:
                    waits = list(si.on_wait)
                    for wcond in waits[:-1]:
                        _ctr[0] += 1
                        nop = mybir.InstNoOp(name=f"waitnop-{_ctr[0]}", ins=[], outs=[])
                        nop.engine = inst.engine
                        nop.sync_info = mybir.SyncInfo(on_wait=[wcond], on_update=[])
                        new.append(nop)
                    inst.sync_info = mybir.SyncInfo(
                        on_wait=[waits[-1]], on_update=list(si.on_update)
                    )
                new.append(inst)
            blk.instructions = new


# quadrant rotations per sel tile: rotation 0 (the squares) is computed on
# DVE as yrep[0:32]*yrep[0:32] (no PE pass, no permuted tile needed — equal
# SBUF base partitions as the hardware requires), freeing its quadrant slot
# for rotation 16 (whose second half is duplicate pairs, weights zeroed).
ROTS = [[16, 1, 2, 3], [4, 5, 6, 7], [8, 9, 10, 11], [12, 13, 14, 15]]


def _host_weights(w1, b1, g1, be1, m1, v1, w2, b2, g2, be2, m2, v2):
    """Precompute device weight layouts on the host."""
    # mm1 lhsT, M-replicated: w1t[k, 32q+c] = w1[c, k]
    w1t = np.zeros((CIN, 128), np.float32)
    for q in range(4):
        w1t[:, 32 * q : 32 * q + 32] = w1.T

    # rotation lhsT: tile j lives at rows 32j..32j+31 of cb1, cols 128j..:
    # perm[32j + k, 128j + 32q + c] = 1 iff k == (c + ROTS[j][q]) % 32
    perm = np.zeros((128, 4 * 128), np.float32)
    for j in range(4):
        for q in range(4):
            r = ROTS[j][q]
            for c in range(32):
                k = (c + r) % 32
                perm[32 * j + k, 128 * j + 32 * q + c] = 1.0

    cb1 = np.zeros((128, CB1_COLS), np.float32)
    cb1[:, CB1_W1A:CB1_W1A + 128] = w1t[0:128]
    cb1[:, CB1_W1B:CB1_W1B + 128] = w1t[128:256]
    cb1[:, CB1_PERM:] = perm

    # w2 permuted into product-row order.
    off = np.zeros(33, np.int64)
    for d in range(32):
        off[d + 1] = off[d] + (32 - d)
    assert off[32] == 528

    def t_index(c, r):
        if c + r < 32:
            d, b_lo = r, c
        else:
            d, b_lo = 32 - r, c + r - 32
        return off[d] + b_lo

    cb2 = np.zeros((128, CB2_COLS), np.float32)
    used = np.zeros(528, bool)
    for j in range(4):
        for q in range(4):
            r = ROTS[j][q]
            for c in range(32):
                if r == 16 and c >= 16:
                    continue  # duplicate half of rotation 16: weights stay 0
                t = t_index(c, r)
                assert not used[t]
                used[t] = True
                cb2[32 * q + c, j * COUT : (j + 1) * COUT] = w2[:, t]
    # squares passes: psq rows 0:32 and 32:64 both hold y_c^2; the m=0
    # chunk contracts rows 0:32 (strip 0), m=1 rows 32:64 (strip 1) so the
    # two sq matmuls overlap each other and the sel matmuls in the array.
    for c in range(32):
        t = t_index(c, 0)
        assert not used[t]
        used[t] = True
        cb2[c, CB2_P16 : CB2_P16 + 128] = w2[0:128, t]
        cb2[32 + c, CB2_P16 + 128 : CB2_P16 + 256] = w2[128:256, t]
    assert used.all()

    inv1 = g1 / np.sqrt(v1 + EPS)
    bn1s = np.tile(inv1, 4).astype(np.float32)
    bn1b = np.tile(b1 * inv1 + be1 - m1 * inv1, 4).astype(np.float32)
    inv2 = g2 / np.sqrt(v2 + EPS)
    bn2s = inv2.reshape(2, 128).T.astype(np.float32)
    bn2b = (b2 * inv2 + be2 - m2 * inv2).reshape(2, 128).T.astype(np.float32)
    bnc = np.zeros((128, 6), np.float32)
    bnc[:, 0] = bn1s
    bnc[:, 1] = bn1b
    bnc[:, 2:4] = bn2s
    bnc[:, 4:6] = bn2b
    return cb1, cb2, bnc


def _build_nc():
    nc = bass.Bass()
    x_d = nc.declare_dram_parameter("x", [IMG_PER_CORE, CIN, NPIX], BF16, isOutput=False)
    cb1_d = nc.declare_dram_parameter("cb1", [128, CB1_COLS], BF16, isOutput=False)
    cb2_d = nc.declare_dram_parameter("cb2", [128, CB2_COLS], BF16, isOutput=False)
    bnc_d = nc.declare_dram_parameter("bnc", [128, 6], F32, isOutput=False)
    out_d = nc.declare_dram_parameter("out", [IMG_PER_CORE, COUT, NPIX], BF16, isOutput=True)

    with tile.TileContext(nc) as tc:
        with (
            tc.tile_pool(name="consts", bufs=1) as cpool,
            tc.tile_pool(name="xp", bufs=1) as xpool,
            tc.tile_pool(name="op", bufs=1) as opool,
            tc.tile_pool(name="yp", bufs=4) as ypool,
            tc.tile_pool(name="y4p", bufs=4) as y4pool,
            tc.tile_pool(name="pp", bufs=16) as ppool,
            tc.tile_pool(name="p16p", bufs=4) as p16pool,
            tc.tile_pool(name="psy", bufs=2, space="PSUM") as psum_y,
            tc.tile_pool(name="pss", bufs=4, space="PSUM") as psum_sel,
            tc.tile_pool(name="psz", bufs=2, space="PSUM") as psum_z,
        ):
            # ---- x tiles: one [128, NPIX] pair per image ----
            xs = {}
            for img in range(IMG_PER_CORE):
                xs[img] = (
                    xpool.tile([128, NPIX], BF16, tag=f"xa{img}", name=f"xa{img}"),
                    xpool.tile([128, NPIX], BF16, tag=f"xb{img}", name=f"xb{img}"),
                )
            # window 0 of image 0 + bn consts ride the ACT hardware DGE
            # (ACT's first instructions, ~1.3us before SP reaches its first
            # DMA); w1/perm is SP's first issue.
            nc.scalar.dma_start(xs[0][0][:, 0:FB], x_d[0, 0:128, 0:FB])
            nc.scalar.dma_start(xs[0][1][:, 0:FB], x_d[0, 128:256, 0:FB])
            bnc = cpool.tile([128, 6], F32, tag="bnc")
            cb1 = cpool.tile([128, CB1_COLS], BF16, tag="cb1")
            cb2 = cpool.tile([128, CB2_COLS], BF16, tag="cb2")
            nc.scalar.dma_start(bnc[:], bnc_d[:])
            nc.sync.dma_start(cb1[:], cb1_d[:])

            # warmup data: iota-generated toggling values (all-constant or
            # all-zero streams draw too little dynamic power to open the
            # HAM clock gate — and measured on this kernel, even the real
            # windows don't: the post-relu product streams are too sparse.
            # Only a sustained ~4-6us run of toggling warmup matmuls trips
            # HAM reliably, so size the warmup accordingly.
            wz = cpool.tile([128, FB], BF16, tag="warmz")
            nc.gpsimd.iota(
                wz[:], [[1, FB]], base=1, channel_multiplier=7,
                allow_small_or_imprecise_dtypes=True,
            )

            # dummy RELU pulls the ~1.3us ACT_TABLE_LOAD into the preamble
            # so it cannot block BN1(w0)
            twsrc = cpool.tile([128, 1], F32, tag="twsrc")
            twarm = cpool.tile([128, 1], F32, tag="tablewarm")
            nc.vector.memset(twsrc[:], 1.0)
            nc.scalar.activation(twarm[:], twsrc[:], AF.Relu)

            # HAM warmup: DMA-free matmuls bridge the gap until x lands
            ps_warm = psum_y.tile([128, FB], F32, tag="psy")
            for _ in range(12):
                nc.tensor.matmul(
                    ps_warm[:], wz[:, 0:128], wz[:], start=True, stop=True
                )

            # rest of the input + w2p, staged by need-time on SP
            nc.sync.dma_start(xs[0][0][:, FB:2 * FB], x_d[0, 0:128, FB:2 * FB])
            nc.sync.dma_start(xs[0][1][:, FB:2 * FB], x_d[0, 128:256, FB:2 * FB])
            nc.sync.dma_start(cb2[:], cb2_d[:])
            nc.sync.dma_start(xs[0][0][:, 2 * FB:4 * FB], x_d[0, 0:128, 2 * FB:4 * FB])
            nc.sync.dma_start(xs[0][1][:, 2 * FB:4 * FB], x_d[0, 128:256, 2 * FB:4 * FB])
            nc.sync.dma_start(xs[0][0][:, 4 * FB:NPIX], x_d[0, 0:128, 4 * FB:NPIX])
            nc.sync.dma_start(xs[0][1][:, 4 * FB:NPIX], x_d[0, 128:256, 4 * FB:NPIX])
            nc.sync.dma_start(xs[1][0][:], x_d[1, 0:128, :])
            nc.sync.dma_start(xs[1][1][:], x_d[1, 128:256, :])

            # whole-image output buffers, one per output-channel chunk
            outb = [
                opool.tile([128, IMG_PER_CORE * NPIX], BF16, tag=f"outb{m}", name=f"outb{m}")
                for m in range(2)
            ]

            def stage_a1(img, win):
                """mm1 + BN1 for one window."""
                s = slice(win * FB, (win + 1) * FB)
                xa, xb = xs[img]
                ps_y = psum_y.tile([128, FB], F32, tag="psy")
                nc.tensor.matmul(
                    ps_y[:], cb1[:, CB1_W1A:CB1_W1A + 128], xa[:, s],
                    start=True, stop=False,
                )
                nc.tensor.matmul(
                    ps_y[:], cb1[:, CB1_W1B:CB1_W1B + 128], xb[:, s],
                    start=False, stop=True,
                )
                yrep = ypool.tile([128, FB], BF16, tag="yrep")
                nc.scalar.activation(
                    yrep[:], ps_y[:], AF.Relu, bias=bnc[:, 1:2], scale=bnc[:, 0:1]
                )
                return yrep

            def stage_sel(yrep):
                """rotations + products for one window."""
                # squares (rotation 0) need no PE pass: same-tile product,
                # both halves of psq hold y^2 (rows 0:32 feed the m=0 sq
                # pass at strip 0, rows 32:64 the m=1 pass at strip 1)
                psq = p16pool.tile([64, FB], BF16, tag="psq")
                nc.gpsimd.tensor_mul(psq[:], yrep[0:64, :], yrep[0:64, :])
                prods = {"sq": psq}
                for j in range(4):
                    ps_sel = psum_sel.tile([128, FB], F32, tag="pssel")
                    nc.tensor.matmul(
                        ps_sel[:],
                        cb1[32 * j : 32 * j + 32, CB1_PERM + 128 * j : CB1_PERM + 128 * (j + 1)],
                        yrep[32 * j : 32 * j + 32, :],
                        start=True,
                        stop=True,
                        tile_position=(32 * j, 0),
                    )
                    pj = ppool.tile([128, FB], BF16, tag="pj")
                    # DVE reads the rotated tile straight from PSUM
                    nc.vector.tensor_mul(pj[:], yrep[:], ps_sel[:])
                    prods[j] = pj
                return prods

            def stage_b_m(img, win, prods, m):
                """mm2 m-chunk: squares pass (strip m) + K=128 passes + BN2."""
                ps_z = psum_z.tile([128, FB], F32, tag="psz")
                nc.tensor.matmul(
                    ps_z[:],
                    cb2[32 * m : 32 * m + 32,
                        CB2_P16 + 128 * m : CB2_P16 + 128 * m + 128],
                    prods["sq"][32 * m : 32 * m + 32, :],
                    start=True, stop=False,
                    tile_position=(32 * m, 0),
                )
                for j in range(4):
                    nc.tensor.matmul(
                        ps_z[:],
                        cb2[:, j * COUT + 128 * m : j * COUT + 128 * m + 128],
                        prods[j][:],
                        start=False, stop=(j == 3),
                    )
                off = img * NPIX + win * FB
                nc.scalar.activation(
                    outb[m][:, off:off + FB], ps_z[:], AF.Relu,
                    bias=bnc[:, 4 + m:5 + m], scale=bnc[:, 2 + m:3 + m],
                )
                fl = _FLUSH.get((img, win, m))
                if fl is not None:
                    a, b = fl
                    nc.sync.dma_start(
                        out_d[img, 128 * m : 128 * m + 128, a:b],
                        outb[m][:, img * NPIX + a : img * NPIX + b],
                    )

            # 2-deep software pipeline; PE stream per window:
            #   mm1(w) | mm2_m0(w-2) | sel(w) | mm2_m1(w-2)
            pipe = []
            for img in range(IMG_PER_CORE):
                for win in range(NWIN):
                    yrep = stage_a1(img, win)
                    if len(pipe) == 2:
                        stage_b_m(*pipe[0], 0)
                    prods = stage_sel(yrep)
                    if len(pipe) == 2:
                        stage_b_m(*pipe.pop(0), 1)
                    pipe.append((img, win, prods))
            for ent in pipe:
                stage_b_m(*ent, 0)
                stage_b_m(*ent, 1)

    _split_multi_waits(nc)
    return nc


_cached = {}


def kernel(**inputs):
    import ml_dtypes

    x = np.asarray(inputs["x"], np.float32)
    args = [
        np.asarray(inputs[k], np.float32)
        for k in ("w1", "b1", "g1", "be1", "m1", "v1", "w2", "b2", "g2", "be2", "m2", "v2")
    ]
    cb1, cb2, bnc = _host_weights(*args)
    cb1 = cb1.astype(ml_dtypes.bfloat16)
    cb2 = cb2.astype(ml_dtypes.bfloat16)

    if "nc" not in _cached:
        _cached["nc"] = _build_nc()
    nc = _cached["nc"]

    xr = x.reshape(B, CIN, NPIX).astype(ml_dtypes.bfloat16)
    shared = {"cb1": cb1, "cb2": cb2, "bnc": bnc}
    in_maps = [
        {"x": np.ascontiguousarray(xr[c * IMG_PER_CORE : (c + 1) * IMG_PER_CORE]), **shared}
        for c in range(N_CORES)
    ]
    res = run_bass_kernel_spmd(nc, in_maps, core_ids=list(range(N_CORES)))
    kernel.last_results = res
    out = np.concatenate(
        [np.asarray(res.results[c]["out"]) for c in range(N_CORES)], axis=0
    ).astype(np.float32)
    return out.reshape(B, COUT, H, W)
